# revision 18
# baseline (speedup 1.0000x reference)
"""Trainium2 Bass kernel: normalized Gaussian spatial convolution.

out[i] = softmax_j( -||x_i - y_j||^2 / (2 sigma^2) ) @ y_fea        (sigma = 0.1)

Shapes: x [1, 12288, 3], y [1, 12288, 3], y_fea [1, 12288, 16] -> out [1, 12288, 16].

Strategy (8 NeuronCores, x sharded along N, y / y_fea replicated):
  Flash-attention-style fusion in a transposed-logit layout.  Per core
  (N_loc = 1536 query points):

  - logits are produced directly by one K=5 matmul with augmented operands:
        S^T[j, i] = x_i . y_j - ||x_i||^2/2 - ||y_j||^2/2  =  -d2/2
    (lhsT = [y; -||y||^2/2; 1], rhs = [x; 1; -||x||^2/2]), so no separate
    distance computation and no per-row bias is needed.
  - P^T = exp(100 * S^T) on the scalar engine (PSUM -> SBUF).  No row-max
    subtraction: logits <= ~0 by construction and the true row max is
    always > -30 for gaussian data, so fp32 exp neither overflows nor
    fully underflows.
  - The denominator is fused as a ones-column in V' = [y_fea, 1]:
        Z = sum_j V'[j] P^T[j, :]   ([17, i] in PSUM, accumulated over
    96 j-chunks, col-packed 2x on the PE array via tile_position).
  - Epilogue: transpose Z chunks with the PE, multiply by 1/denominator,
    DMA out.

  j-chunk c (c = 0..95) is the non-contiguous set {j = 96*p + c}, which
  makes every y-side DMA contiguous per partition.  The i (query) order
  inside a core is i' = a*128 + q  <->  x row 12*q + a; the output DMA
  un-permutes, so DRAM out is in natural row order.
"""

import sys

import numpy as np

for _p in ("/opt/trn_rl_repo",):
    if _p not in sys.path:
        sys.path.insert(0, _p)

import os  # noqa: E402

import concourse.bass as bass  # noqa: E402
import concourse.tile as tile  # noqa: E402
from concourse import bacc, mybir  # noqa: E402
from concourse.bass_utils import run_bass_kernel_spmd  # noqa: E402
from concourse.masks import make_identity  # noqa: E402

F32 = mybir.dt.float32
EXP = mybir.ActivationFunctionType.Exp

N_CORES = 8
N = 12288
M = 12288
D = 16
NL = N // N_CORES          # 1536 query points per core
SIGMA = 0.1
INV_S2 = 1.0 / (SIGMA * SIGMA)   # exp(INV_S2 * m), m = -d2/2

# debug/bisection knobs.  tile_position col-packing (GK_COLPACK=1) crashes the
# NRT on this toolchain, so it stays off; row-packing of mm1 is controlled by
# GK_ROWPACK.
COLPACK = os.environ.get("GK_COLPACK", "0") == "1"
EXP_SPLIT = os.environ.get("GK_EXP_SPLIT", "0") == "1"
ROWPACK = os.environ.get("GK_ROWPACK", "1") == "1"

PJ = M // 128              # 96 j's per partition; chunk c = {j = PJ*p + c}
NCH = M // 128             # 96 chunks of 128 j's
PI = NL // 128             # 12 i's per partition in the x-norm layout
ITILE = 512                # matmul moving free dim (fp32 max / 1 PSUM bank)
NIT = NL // ITILE          # 3 i-tiles
PAIR = NCH // 2            # 48 chunk-pairs per i-tile
DV = D + 1                 # V' columns (y_fea ++ ones)


def _build_program():
    nc = bacc.Bacc(
        "TRN2",
        target_bir_lowering=False,
        debug=False,
        num_devices=N_CORES,
    )

    x_d = nc.dram_tensor("x", [NL, 3], F32, kind="ExternalInput")
    y_d = nc.dram_tensor("y", [M, 3], F32, kind="ExternalInput")
    yf_d = nc.dram_tensor("yf", [M, D], F32, kind="ExternalInput")
    out_d = nc.dram_tensor("out", [NL, D], F32, kind="ExternalOutput")

    x_ap = x_d.ap()
    y_ap = y_d.ap()
    yf_ap = yf_d.ap()
    # out rows: i = PI*q + b  <->  free index i' = b*128 + q
    outv = out_d.ap().rearrange("(q b) d -> q b d", q=128)

    with tile.TileContext(nc) as tc:
        with (
            tc.tile_pool(name="singles", bufs=1) as singles,
            tc.tile_pool(name="ppool", bufs=5) as ppool,
            tc.tile_pool(name="outp", bufs=2) as outp,
            tc.tile_pool(name="small", bufs=4) as small,
            tc.tile_pool(name="spool", bufs=2, space="PSUM") as spool,
            tc.tile_pool(name="ztpool", bufs=4, space="PSUM") as ztpool,
        ):
            idn = singles.tile([128, 128], F32)
            make_identity(nc, idn[:])

            ones_sb = singles.tile([128, 128], F32)
            nc.vector.memset(ones_sb[:], 1.0)

            # ---- V' = [y_fea, 1] in chunk layout: vt[p, c, 0:16], vt[p, c, 16] = 1
            vt = singles.tile([128, PJ, DV], F32)
            nc.vector.memset(vt[:, :, D : D + 1], 1.0)
            yf_v = yf_ap.rearrange("(p a) d -> p a d", p=128)
            for piece in range(8):
                c0 = piece * (PJ // 8)
                c1 = c0 + PJ // 8
                eng = nc.sync if piece % 2 == 0 else nc.scalar
                eng.dma_start(out=vt[:, c0:c1, 0:D], in_=yf_v[:, c0:c1, :])

            def row_via_transpose(dst_row, src, width):
                """dst_row[0, a, p] = src[p, a] via PE transpose + flatten DMA.

                src is [128, width] (possibly strided), dst_row [1, width, 128].
                """
                if src.ap[-1][0] != 1:
                    # PE transpose wants a contiguous stationary operand.
                    dense = small.tile([128, 128], F32, tag="dense")
                    nc.vector.tensor_copy(dense[:, 0:width], src)
                    src = dense[:, 0:width]
                t_ps = ztpool.tile([128, 512], F32, tag="zt")
                nc.tensor.transpose(t_ps[0:width, 0:128], src, idn[:])
                t_sb = small.tile([128, 128], F32, tag="tcp")
                nc.vector.tensor_copy(t_sb[0:width, :], t_ps[0:width, 0:128])
                nc.sync.dma_start(out=dst_row, in_=t_sb[0:width, :])

            # ---- y side: yt[p, a, c] = y[PJ*p + a, c]  (contiguous DMA)
            yt = singles.tile([128, PJ, 3], F32)
            nc.sync.dma_start(out=yt[:], in_=y_ap.rearrange("(p a) c -> p a c", p=128))
            ysq = singles.tile([128, PJ, 3], F32)
            nc.vector.tensor_mul(ysq[:], yt[:], yt[:])
            yn_a = singles.tile([128, PJ], F32)
            nc.vector.tensor_add(yn_a[:], ysq[:, :, 0], ysq[:, :, 1])
            yn = singles.tile([128, PJ], F32)
            nc.vector.tensor_add(yn[:], yn_a[:], ysq[:, :, 2])
            ynh = singles.tile([128, PJ], F32)
            nc.vector.tensor_scalar_mul(ynh[:], yn[:], -0.5)

            # ---- Y5 stationary [5, (c p)]: rows y0,y1,y2, -||y||^2/2, 1
            # With ROWPACK a second copy lives at partitions 32..36 so two
            # chunks can run concurrently in different PE row groups.
            y5 = singles.tile([37 if ROWPACK else 5, NCH, 128], F32)
            ybases = (0, 32) if ROWPACK else (0,)
            for b in ybases:
                for k in range(3):
                    row_via_transpose(y5[b + k : b + k + 1], yt[:, :, k], PJ)
                row_via_transpose(y5[b + 3 : b + 4], ynh[:], PJ)
                nc.sync.dma_start(out=y5[b + 4 : b + 5], in_=ones_sb[0:PJ, :])

            # ---- x side (12 wide)
            xt = singles.tile([128, PI, 3], F32)
            nc.sync.dma_start(out=xt[:], in_=x_ap.rearrange("(p a) c -> p a c", p=128))
            xsq = singles.tile([128, PI, 3], F32)
            nc.vector.tensor_mul(xsq[:], xt[:], xt[:])
            xn_a = singles.tile([128, PI], F32)
            nc.vector.tensor_add(xn_a[:], xsq[:, :, 0], xsq[:, :, 1])
            xn = singles.tile([128, PI], F32)
            nc.vector.tensor_add(xn[:], xn_a[:], xsq[:, :, 2])
            xnh = singles.tile([128, PI], F32)
            nc.vector.tensor_scalar_mul(xnh[:], xn[:], -0.5)

            # ---- X5 moving operand [5, (a q)]: rows x0,x1,x2, 1, -||x||^2/2
            x5 = singles.tile([37 if ROWPACK else 5, PI, 128], F32)
            for b in ybases:
                for k in range(3):
                    row_via_transpose(x5[b + k : b + k + 1], xt[:, :, k], PI)
                nc.sync.dma_start(out=x5[b + 3 : b + 4], in_=ones_sb[0:PI, :])
                row_via_transpose(x5[b + 4 : b + 5], xnh[:], PI)

            # ---- main fused loop, software-pipelined emission
            s_tiles = {}
            p_tiles = {}
            z_tiles = {}
            NGLOB = NIT * PAIR

            def emit_mm1(g):
                it, t = divmod(g, PAIR)
                s = spool.tile([128, 1024], F32, tag="s")
                s_tiles[g] = s
                for h in (0, 1):
                    c = 2 * t + h
                    b = 32 * h if ROWPACK else 0
                    nc.tensor.matmul(
                        s[:, 512 * h : 512 * (h + 1)],
                        y5[b : b + 5, c, :],
                        x5[b : b + 5, 4 * it : 4 * it + 4, :],
                        start=True,
                        stop=True,
                        tile_position=(b, 0) if ROWPACK else None,
                    )

            ZB0 = 32 if COLPACK else 0  # zB output base partition

            def emit_exp(g):
                s = s_tiles.pop(g)
                p = ppool.tile([128, 1024], F32, tag="p")
                p_tiles[g] = p
                if EXP_SPLIT:
                    nc.scalar.activation(
                        p[:, 0:512], s[:, 0:512], EXP, bias=0.0, scale=INV_S2
                    )
                    nc.scalar.activation(
                        p[:, 512:1024], s[:, 512:1024], EXP, bias=0.0, scale=INV_S2
                    )
                else:
                    nc.scalar.activation(p[:], s[:], EXP, bias=0.0, scale=INV_S2)

            def emit_mm2(g):
                it, t = divmod(g, PAIR)
                zA, zB = z_tiles[it]
                p = p_tiles.pop(g)
                nc.tensor.matmul(
                    zA[0:DV, :],
                    vt[:, 2 * t, :],
                    p[:, 0:512],
                    start=(t == 0),
                    stop=(t == PAIR - 1),
                    tile_position=(0, 0) if COLPACK else None,
                )
                nc.tensor.matmul(
                    zB[ZB0 : ZB0 + DV, :],
                    vt[:, 2 * t + 1, :],
                    p[:, 512:1024],
                    start=(t == 0),
                    stop=(t == PAIR - 1),
                    tile_position=(0, 32) if COLPACK else None,
                )

            def emit_epiA(it):
                zA, zB = z_tiles.pop(it)
                zs = small.tile([32 + DV, 512], F32, tag="zs")
                zsb = small.tile([32 + DV, 512], F32, tag="zsb")
                nc.vector.tensor_copy(zs[0:DV, :], zA[0:DV, :])
                nc.vector.tensor_copy(
                    zsb[ZB0 : ZB0 + DV, :], zB[ZB0 : ZB0 + DV, :]
                )
                return zs, zsb

            def emit_epiB(it, zs, zsb):
                tps = ztpool.tile([128, 512], F32, tag="zt")
                osb = outp.tile([128, 4, D], F32, tag="osb")
                for k in range(4):
                    off = 2 * DV * k
                    nc.tensor.transpose(
                        tps[:, off : off + DV],
                        zs[0:DV, 128 * k : 128 * (k + 1)],
                        idn[0:DV, 0:DV],
                    )
                    nc.tensor.transpose(
                        tps[:, off + DV : off + 2 * DV],
                        zsb[ZB0 : ZB0 + DV, 128 * k : 128 * (k + 1)],
                        idn[ZB0 : ZB0 + DV, ZB0 : ZB0 + DV],
                    )
                tsb = small.tile([128, 8 * DV], F32, tag="tsb")
                nc.vector.tensor_copy(tsb[:], tps[:, 0 : 8 * DV])
                for k in range(4):
                    off = 2 * DV * k
                    num = small.tile([128, D], F32, tag="num")
                    den = small.tile([128, 1], F32, tag="den")
                    nc.vector.tensor_add(
                        num[:], tsb[:, off : off + D], tsb[:, off + DV : off + DV + D]
                    )
                    nc.vector.tensor_add(
                        den[:],
                        tsb[:, off + D : off + DV],
                        tsb[:, off + DV + D : off + 2 * DV],
                    )
                    rec = small.tile([128, 1], F32, tag="rec")
                    nc.vector.reciprocal(rec[:], den[:])
                    nc.vector.tensor_scalar_mul(osb[:, k, :], num[:], rec[:])
                nc.sync.dma_start(out=outv[:, 4 * it : 4 * it + 4, :], in_=osb[:])

            pendingB = None
            emit_mm1(0)
            for g in range(NGLOB):
                it, t = divmod(g, PAIR)
                if t == 0:
                    zA = ztpool.tile([128, 512], F32, tag="zt")
                    zB = ztpool.tile([128, 512], F32, tag="zt")
                    z_tiles[it] = (zA, zB)
                if g + 1 < NGLOB:
                    emit_mm1(g + 1)
                if pendingB is not None and t == 3:
                    emit_epiB(*pendingB)
                    pendingB = None
                emit_exp(g)
                emit_mm2(g)
                if t == PAIR - 1:
                    zs, zsb = emit_epiA(it)
                    pendingB = (it, zs, zsb)
            if pendingB is not None:
                emit_epiB(*pendingB)

    nc.compile()
    return nc


_CACHE = {}


def _get_program():
    if "nc" not in _CACHE:
        _CACHE["nc"] = _build_program()
    return _CACHE["nc"]


def _prep_inputs(x, y, y_fea):
    x = np.ascontiguousarray(np.asarray(x, dtype=np.float32)).reshape(N, 3)
    y = np.ascontiguousarray(np.asarray(y, dtype=np.float32)).reshape(M, 3)
    yf = np.ascontiguousarray(np.asarray(y_fea, dtype=np.float32)).reshape(M, D)
    return [
        {"x": x[c * NL : (c + 1) * NL], "y": y, "yf": yf} for c in range(N_CORES)
    ]


def run_spmd(x, y, y_fea, **kwargs):
    """Run on the 8 cores; returns (out [1,N,D], BassKernelResults)."""
    nc = _get_program()
    in_maps = _prep_inputs(x, y, y_fea)
    res = run_bass_kernel_spmd(nc, in_maps, list(range(N_CORES)), **kwargs)
    outs = [np.asarray(res.results[c]["out"]) for c in range(N_CORES)]
    out = np.concatenate(outs, axis=0).reshape(1, N, D).astype(np.float32)
    return out, res


def kernel(x, y, y_fea):
    out, _ = run_spmd(x, y, y_fea)
    return out


if __name__ == "__main__":
    _get_program()
    print("program built OK")


# revision 24
# speedup vs baseline: 1.3846x; 1.3846x over previous
"""Trainium2 Bass kernel: normalized Gaussian spatial convolution.

out[i] = softmax_j( -||x_i - y_j||^2 / (2 sigma^2) ) @ y_fea        (sigma = 0.1)

Shapes: x [1, 12288, 3], y [1, 12288, 3], y_fea [1, 12288, 16] -> out [1, 12288, 16].

Strategy (8 NeuronCores, x sharded along N, y / y_fea replicated):
  Flash-attention-style fusion in a transposed-logit layout.  Per core
  (N_loc = 1536 query points):

  - logits are produced directly by one K=5 matmul with augmented operands:
        S^T[j, i] = x_i . y_j - ||x_i||^2/2 - ||y_j||^2/2  =  -d2/2
    (lhsT = [y; -||y||^2/2; 1], rhs = [x; 1; -||x||^2/2]), so no separate
    distance computation and no per-row bias is needed.
  - P^T = exp(100 * S^T) on the scalar engine (PSUM -> SBUF).  No row-max
    subtraction: logits <= ~0 by construction and the true row max is
    always > -30 for gaussian data, so fp32 exp neither overflows nor
    fully underflows.
  - The denominator is fused as a ones-column in V' = [y_fea, 1]:
        Z = sum_j V'[j] P^T[j, :]   ([17, i] in PSUM, accumulated over
    96 j-chunks, col-packed 2x on the PE array via tile_position).
  - Epilogue: transpose Z chunks with the PE, multiply by 1/denominator,
    DMA out.

  j-chunk c (c = 0..95) is the non-contiguous set {j = 96*p + c}, which
  makes every y-side DMA contiguous per partition.  The i (query) order
  inside a core is i' = a*128 + q  <->  x row 12*q + a; the output DMA
  un-permutes, so DRAM out is in natural row order.
"""

import sys

import numpy as np

for _p in ("/opt/trn_rl_repo",):
    if _p not in sys.path:
        sys.path.insert(0, _p)

import os  # noqa: E402

import concourse.bass as bass  # noqa: E402
import concourse.tile as tile  # noqa: E402
from concourse import bacc, mybir  # noqa: E402
from concourse.bass_utils import run_bass_kernel_spmd  # noqa: E402
from concourse.masks import make_identity  # noqa: E402

F32 = mybir.dt.float32
EXP = mybir.ActivationFunctionType.Exp

N_CORES = 8
N = 12288
M = 12288
D = 16
NL = N // N_CORES          # 1536 query points per core
SIGMA = 0.1
INV_S2 = 1.0 / (SIGMA * SIGMA)   # exp(INV_S2 * m), m = -d2/2

# debug/bisection knobs.  tile_position col-packing (GK_COLPACK=1) crashes the
# NRT on this toolchain, so it stays off; row-packing of mm1 is controlled by
# GK_ROWPACK.
COLPACK = os.environ.get("GK_COLPACK", "0") == "1"
EXP_SPLIT = os.environ.get("GK_EXP_SPLIT", "0") == "1"
ROWPACK = os.environ.get("GK_ROWPACK", "1") == "1"

PJ = M // 128              # 96 j's per partition; chunk c = {j = PJ*p + c}
NCH = M // 128             # 96 chunks of 128 j's
PI = NL // 128             # 12 i's per partition in the x-norm layout
ITILE = 512                # matmul moving free dim (fp32 max / 1 PSUM bank)
NIT = NL // ITILE          # 3 i-tiles
TRI = 3                    # chunks per exp group (3 PSUM banks per s tile)
NG = NCH // TRI            # 32 chunk-groups per i-tile
DV = D + 1                 # V' columns (y_fea ++ ones)


def _build_program():
    nc = bacc.Bacc(
        "TRN2",
        target_bir_lowering=False,
        debug=False,
        num_devices=N_CORES,
    )

    x_d = nc.dram_tensor("x", [NL, 3], F32, kind="ExternalInput")
    y_d = nc.dram_tensor("y", [M, 3], F32, kind="ExternalInput")
    yf_d = nc.dram_tensor("yf", [M, D], F32, kind="ExternalInput")
    out_d = nc.dram_tensor("out", [NL, D], F32, kind="ExternalOutput")

    x_ap = x_d.ap()
    y_ap = y_d.ap()
    yf_ap = yf_d.ap()
    # out rows: i = PI*q + b  <->  free index i' = b*128 + q
    outv = out_d.ap().rearrange("(q b) d -> q b d", q=128)

    with tile.TileContext(nc) as tc:
        with (
            tc.tile_pool(name="singles", bufs=1) as singles,
            tc.tile_pool(name="ppool", bufs=5) as ppool,
            tc.tile_pool(name="outp", bufs=2) as outp,
            tc.tile_pool(name="small", bufs=4) as small,
            tc.tile_pool(name="spool", bufs=2, space="PSUM") as spool,
            tc.tile_pool(name="ztpool", bufs=2, space="PSUM") as ztpool,
        ):
            idn = singles.tile([128, 128], F32)
            make_identity(nc, idn[:])

            ones_sb = singles.tile([128, 128], F32)
            nc.vector.memset(ones_sb[:], 1.0)

            # ---- V' = [y_fea, 1] in chunk layout: vt[p, c, 0:16], vt[p, c, 16] = 1
            vt = singles.tile([128, PJ, DV], F32)
            nc.vector.memset(vt[:, :, D : D + 1], 1.0)
            yf_v = yf_ap.rearrange("(p a) d -> p a d", p=128)
            for piece in range(8):
                c0 = piece * (PJ // 8)
                c1 = c0 + PJ // 8
                eng = nc.sync if piece % 2 == 0 else nc.scalar
                eng.dma_start(out=vt[:, c0:c1, 0:D], in_=yf_v[:, c0:c1, :])

            def row_via_transpose(dst_row, src, width):
                """dst_row[0, a, p] = src[p, a] via PE transpose + flatten DMA.

                src is [128, width] (possibly strided), dst_row [1, width, 128].
                """
                if src.ap[-1][0] != 1:
                    # PE transpose wants a contiguous stationary operand.
                    dense = small.tile([128, 128], F32, tag="dense")
                    nc.vector.tensor_copy(dense[:, 0:width], src)
                    src = dense[:, 0:width]
                t_ps = ztpool.tile([128, 512], F32, tag="zt")
                nc.tensor.transpose(t_ps[0:width, 0:128], src, idn[:])
                t_sb = small.tile([128, 128], F32, tag="tcp")
                nc.vector.tensor_copy(t_sb[0:width, :], t_ps[0:width, 0:128])
                nc.sync.dma_start(out=dst_row, in_=t_sb[0:width, :])

            # ---- y side: yt[p, a, c] = y[PJ*p + a, c]  (contiguous DMA)
            yt = singles.tile([128, PJ, 3], F32)
            nc.sync.dma_start(out=yt[:], in_=y_ap.rearrange("(p a) c -> p a c", p=128))
            ysq = singles.tile([128, PJ, 3], F32)
            nc.vector.tensor_mul(ysq[:], yt[:], yt[:])
            yn_a = singles.tile([128, PJ], F32)
            nc.vector.tensor_add(yn_a[:], ysq[:, :, 0], ysq[:, :, 1])
            yn = singles.tile([128, PJ], F32)
            nc.vector.tensor_add(yn[:], yn_a[:], ysq[:, :, 2])
            ynh = singles.tile([128, PJ], F32)
            nc.vector.tensor_scalar_mul(ynh[:], yn[:], -0.5)

            # ---- Y5 stationary [5, (c p)]: rows y0,y1,y2, -||y||^2/2, 1
            # With ROWPACK a second copy lives at partitions 32..36 so two
            # chunks can run concurrently in different PE row groups.
            y5 = singles.tile([69 if ROWPACK else 5, NCH, 128], F32)
            ybases = (0, 32, 64) if ROWPACK else (0,)
            for b in ybases:
                for k in range(3):
                    row_via_transpose(y5[b + k : b + k + 1], yt[:, :, k], PJ)
                row_via_transpose(y5[b + 3 : b + 4], ynh[:], PJ)
                nc.sync.dma_start(out=y5[b + 4 : b + 5], in_=ones_sb[0:PJ, :])

            # ---- x side (12 wide)
            xt = singles.tile([128, PI, 3], F32)
            nc.sync.dma_start(out=xt[:], in_=x_ap.rearrange("(p a) c -> p a c", p=128))
            xsq = singles.tile([128, PI, 3], F32)
            nc.vector.tensor_mul(xsq[:], xt[:], xt[:])
            xn_a = singles.tile([128, PI], F32)
            nc.vector.tensor_add(xn_a[:], xsq[:, :, 0], xsq[:, :, 1])
            xn = singles.tile([128, PI], F32)
            nc.vector.tensor_add(xn[:], xn_a[:], xsq[:, :, 2])
            xnh = singles.tile([128, PI], F32)
            nc.vector.tensor_scalar_mul(xnh[:], xn[:], -0.5)

            # ---- X5 moving operand [5, (a q)]: rows x0,x1,x2, 1, -||x||^2/2
            x5 = singles.tile([69 if ROWPACK else 5, PI, 128], F32)
            for b in ybases:
                for k in range(3):
                    row_via_transpose(x5[b + k : b + k + 1], xt[:, :, k], PI)
                nc.sync.dma_start(out=x5[b + 3 : b + 4], in_=ones_sb[0:PI, :])
                row_via_transpose(x5[b + 4 : b + 5], xnh[:], PI)

            # ---- main fused loop, software-pipelined emission
            # Groups of TRI=3 chunks: one s tile spans 3 PSUM banks so each
            # exp instruction covers [128, 1536]; both mm2 streams accumulate
            # into a single zA (serial on PE anyway without col-packing).
            s_tiles = {}
            p_tiles = {}
            z_tiles = {}
            NGLOB = NIT * NG

            def emit_mm1(g):
                it, t = divmod(g, NG)
                s = spool.tile([128, TRI * 512], F32, tag="s")
                s_tiles[g] = s
                for h in range(TRI):
                    c = TRI * t + h
                    b = (0, 32, 64)[h] if ROWPACK else 0
                    nc.tensor.matmul(
                        s[:, 512 * h : 512 * (h + 1)],
                        y5[b : b + 5, c, :],
                        x5[b : b + 5, 4 * it : 4 * it + 4, :],
                        start=True,
                        stop=True,
                        tile_position=(b, 0) if ROWPACK else None,
                    )

            def emit_exp(g):
                s = s_tiles.pop(g)
                p = ppool.tile([128, TRI * 512], F32, tag="p")
                p_tiles[g] = p
                if EXP_SPLIT:
                    for h in range(TRI):
                        nc.scalar.activation(
                            p[:, 512 * h : 512 * (h + 1)],
                            s[:, 512 * h : 512 * (h + 1)],
                            EXP,
                            bias=0.0,
                            scale=INV_S2,
                        )
                else:
                    nc.scalar.activation(p[:], s[:], EXP, bias=0.0, scale=INV_S2)

            def emit_mm2(g):
                it, t = divmod(g, NG)
                zA = z_tiles[it]
                p = p_tiles.pop(g)
                for h in range(TRI):
                    nc.tensor.matmul(
                        zA[0:DV, :],
                        vt[:, TRI * t + h, :],
                        p[:, 512 * h : 512 * (h + 1)],
                        start=(t == 0 and h == 0),
                        stop=(t == NG - 1 and h == TRI - 1),
                    )

            def emit_epiA(it):
                zA = z_tiles.pop(it)
                zs = small.tile([DV, 512], F32, tag="zs")
                nc.vector.tensor_copy(zs[:], zA[0:DV, :])
                return zs

            def emit_epiB(it, zs):
                tps = ztpool.tile([128, 512], F32, tag="zt")
                osb = outp.tile([128, 4, D], F32, tag="osb")
                for k in range(4):
                    nc.tensor.transpose(
                        tps[:, DV * k : DV * (k + 1)],
                        zs[:, 128 * k : 128 * (k + 1)],
                        idn[0:DV, 0:DV],
                    )
                tsb = small.tile([128, 4 * DV], F32, tag="tsb")
                nc.vector.tensor_copy(tsb[:], tps[:, 0 : 4 * DV])
                for k in range(4):
                    off = DV * k
                    rec = small.tile([128, 1], F32, tag="rec")
                    nc.vector.reciprocal(rec[:], tsb[:, off + D : off + DV])
                    nc.vector.tensor_scalar_mul(
                        osb[:, k, :], tsb[:, off : off + D], rec[:]
                    )
                nc.sync.dma_start(out=outv[:, 4 * it : 4 * it + 4, :], in_=osb[:])

            pendingB = None
            emit_mm1(0)
            for g in range(NGLOB):
                it, t = divmod(g, NG)
                if t == 0:
                    zA = ztpool.tile([128, 512], F32, tag="zt")
                    z_tiles[it] = zA
                if g + 1 < NGLOB:
                    emit_mm1(g + 1)
                if pendingB is not None and t == 3:
                    emit_epiB(*pendingB)
                    pendingB = None
                emit_exp(g)
                emit_mm2(g)
                if t == NG - 1:
                    pendingB = (it, emit_epiA(it))
            if pendingB is not None:
                emit_epiB(*pendingB)

    nc.compile()
    return nc


_CACHE = {}


def _get_program():
    if "nc" not in _CACHE:
        _CACHE["nc"] = _build_program()
    return _CACHE["nc"]


def _prep_inputs(x, y, y_fea):
    x = np.ascontiguousarray(np.asarray(x, dtype=np.float32)).reshape(N, 3)
    y = np.ascontiguousarray(np.asarray(y, dtype=np.float32)).reshape(M, 3)
    yf = np.ascontiguousarray(np.asarray(y_fea, dtype=np.float32)).reshape(M, D)
    return [
        {"x": x[c * NL : (c + 1) * NL], "y": y, "yf": yf} for c in range(N_CORES)
    ]


def run_spmd(x, y, y_fea, **kwargs):
    """Run on the 8 cores; returns (out [1,N,D], BassKernelResults)."""
    nc = _get_program()
    in_maps = _prep_inputs(x, y, y_fea)
    res = run_bass_kernel_spmd(nc, in_maps, list(range(N_CORES)), **kwargs)
    outs = [np.asarray(res.results[c]["out"]) for c in range(N_CORES)]
    out = np.concatenate(outs, axis=0).reshape(1, N, D).astype(np.float32)
    return out, res


def kernel(x, y, y_fea):
    out, _ = run_spmd(x, y, y_fea)
    return out


if __name__ == "__main__":
    _get_program()
    print("program built OK")
